# revision 1
# baseline (speedup 1.0000x reference)
"""RNN-T JointNetwork kernel for 8 Trainium2 NeuronCores (raw bass).

reference:
  e = enc @ W_enc.T + b_enc          # [B,T,H]
  d = dec @ W_dec.T + b_dec          # [B,U,H]
  j = tanh(e[:,:,None,:] + d[:,None,:,:])
  out = j @ W_joint.T + b_joint      # [B,T,U,V]

Sharding: T (256) split 8 ways -> 32 t-rows per core; host concatenates
along T.

Per-core dataflow (H on partitions for everything left of the big matmul):
  PE:  E^T[h, m] and D^T[h, n] projections, then per (b,t) row m the vocab
       matmul psum[u, v] += Jt[h, u]^T W_joint^T[h, v] (4 k-tiles x 2 v-banks)
  ACT: Jt[h, u] = tanh(D^T[h, (b,u)] + E^T[h, m]) via per-partition bias port
  DVE: drains psum -> sbuf while adding replicated b_joint
  SP:  all DMA (inputs once, one 512KB contiguous output row per m)

This toolchain's walrus rejects any compute instruction carrying >=2 sync
waits, so the kernel is written in raw bass: all cross-engine waits are
standalone wait_ge instructions and compute instructions carry none.
"""

import numpy as np

B, T, U = 4, 256, 128
ENC_DIM = DEC_DIM = HID = 512
VOCAB = 1024
NCORES = 8
TC = T // NCORES        # 32 t-rows per core
M = B * TC              # 128 (b,t) rows per core
KT = HID // 128         # 4 contraction tiles
HT = HID // 128         # 4 h tiles
NJT = 4                 # jt double-buffers
NOT = 6                 # output staging buffers
NPS = 4                 # psum tiles in flight (8 banks total)

_CACHE = {}


def _build_bass(reps=1, timing=False):
    import concourse.bass as bass
    import concourse.mybir as mybir

    f32 = mybir.dt.float32
    f32r = mybir.dt.float32r
    Tanh = mybir.ActivationFunctionType.Tanh

    nc = bass.Bass()
    encT = nc.declare_dram_parameter("encT", [128, KT, M], f32, isOutput=False)
    decT = nc.declare_dram_parameter("decT", [128, KT, B * U], f32, isOutput=False)
    WencT = nc.declare_dram_parameter("WencT", [128, KT, HID], f32, isOutput=False)
    WdecT = nc.declare_dram_parameter("WdecT", [128, KT, HID], f32, isOutput=False)
    WjT = nc.declare_dram_parameter("WjT", [128, HT, VOCAB], f32, isOutput=False)
    bsum = nc.declare_dram_parameter("bsum", [128, HT], f32, isOutput=False)
    bj = nc.declare_dram_parameter("bj", [128, VOCAB], f32, isOutput=False)
    if timing:
        out = nc.dram_tensor("out_i", [M, U, VOCAB], f32)
        tok = nc.declare_dram_parameter("tok", [128, 4], f32, isOutput=True)
    else:
        out = nc.declare_dram_parameter("out", [M, U, VOCAB], f32, isOutput=True)

    N_IN_DMA = 7

    from contextlib import ExitStack

    with ExitStack() as ctx:
        e = ctx.enter_context
        encT_sb = e(nc.sbuf_tensor("encT_sb", [128, KT, M], f32))
        decT_sb = e(nc.sbuf_tensor("decT_sb", [128, KT, B * U], f32))
        WencT_sb = e(nc.sbuf_tensor("WencT_sb", [128, KT, HID], f32))
        WdecT_sb = e(nc.sbuf_tensor("WdecT_sb", [128, KT, HID], f32))
        WjT_stage = e(nc.sbuf_tensor("WjT_stage", [128, HT, VOCAB], f32))
        WjT_sb = e(nc.sbuf_tensor("WjT_sb", [128, HT, VOCAB], f32r))
        bsum_sb = e(nc.sbuf_tensor("bsum_sb", [128, HT], f32))
        bj_sb = e(nc.sbuf_tensor("bj_sb", [128, VOCAB], f32))
        ET_sb = e(nc.sbuf_tensor("ET_sb", [128, HT, M], f32))
        DT_sb = e(nc.sbuf_tensor("DT_sb", [128, HT, B * U], f32))
        jt_sb = e(nc.sbuf_tensor("jt_sb", [128, NJT, HT, 128], f32r))
        ot_sb = e(nc.sbuf_tensor("ot_sb", [128, NOT, VOCAB], f32))
        ps = [
            e(nc.psum_tensor(f"ps{i}", [128, VOCAB], f32)) for i in range(NPS)
        ]
        s_in = e(nc.semaphore("s_in"))
        s_act = e(nc.semaphore("s_act"))
        s_pe = e(nc.semaphore("s_pe"))
        s_dve = e(nc.semaphore("s_dve"))
        s_outd = e(nc.semaphore("s_outd"))
        block = e(nc.Block())

        @block.sync
        def _(sync):
            for sb, dr in (
                (encT_sb, encT),
                (decT_sb, decT),
                (WencT_sb, WencT),
                (WdecT_sb, WdecT),
                (WjT_stage, WjT),
                (bsum_sb, bsum),
                (bj_sb, bj),
            ):
                sync.dma_start(out=sb[:], in_=dr[:]).then_inc(s_in, 16)
            for rep in range(reps):
                for m in range(M):
                    c = rep * M + m
                    sync.wait_ge(s_dve, 9 + c + 1)
                    sync.dma_start(out=out[m], in_=ot_sb[:, c % NOT, :]).then_inc(
                        s_outd, 16
                    )
            sync.wait_ge(s_outd, 16 * M * reps)
            if timing:
                sync.dma_start(out=tok[:], in_=bsum_sb[:]).then_inc(s_in, 16)
                sync.wait_ge(s_in, 16 * (N_IN_DMA + 1))

        @block.tensor
        def _(pe):
            pe.wait_ge(s_in, 16 * N_IN_DMA)
            # E^T: ps[hi][:, 0:M] (bank 2*hi)
            for hi in range(HT):
                for ki in range(KT):
                    mm = pe.matmul(
                        ps[hi][:, 0:M],
                        WencT_sb[:, ki, hi * 128 : (hi + 1) * 128],
                        encT_sb[:, ki, :],
                        start=(ki == 0),
                        stop=(ki == KT - 1),
                    )
                mm.then_inc(s_pe, 1)
            # D^T: ps[hi][:, 512:1024] (bank 2*hi+1)
            for hi in range(HT):
                for ki in range(KT):
                    mm = pe.matmul(
                        ps[hi][:, 512 : 512 + B * U],
                        WdecT_sb[:, ki, hi * 128 : (hi + 1) * 128],
                        decT_sb[:, ki, :],
                        start=(ki == 0),
                        stop=(ki == KT - 1),
                    )
                mm.then_inc(s_pe, 1)
            # main loop: s_pe = 8 + c + 1 after group c
            pe.wait_ge(s_dve, 9)  # WjT f32r cast done
            for rep in range(reps):
                for m in range(M):
                    c = rep * M + m
                    pe.wait_ge(s_act, 4 * (c + 1))
                    pe.wait_ge(s_dve, c + 6)  # psum slot free
                    for hi in range(HT):
                        for vi in range(2):
                            mm = pe.matmul(
                                ps[c % NPS][:, vi * 512 : (vi + 1) * 512],
                                jt_sb[:, c % NJT, hi, :],
                                WjT_sb[:, hi, vi * 512 : (vi + 1) * 512],
                                start=(hi == 0),
                                stop=(hi == HT - 1),
                            )
                    mm.then_inc(s_pe, 1)

        @block.scalar
        def _(act):
            act.wait_ge(s_dve, 8)  # ET/DT ready
            for rep in range(reps):
                for m in range(M):
                    c = rep * M + m
                    b = m // TC
                    if c >= NJT:
                        act.wait_ge(s_pe, 8 + (c - NJT) + 1)  # jt slot free
                    for hi in range(HT):
                        act.activation(
                            jt_sb[:, c % NJT, hi, :],
                            DT_sb[:, hi, b * 128 : (b + 1) * 128],
                            Tanh,
                            bias=ET_sb[:, hi, m : m + 1],
                        ).then_inc(s_act, 1)

        @block.vector
        def _(dve):
            dve.wait_ge(s_in, 16 * N_IN_DMA)
            for hi in range(HT):
                dve.wait_ge(s_pe, hi + 1)
                dve.tensor_copy(ET_sb[:, hi, :], ps[hi][:, 0:M]).then_inc(s_dve, 1)
            for hi in range(HT):
                dve.wait_ge(s_pe, 4 + hi + 1)
                dve.tensor_scalar_add(
                    DT_sb[:, hi, :],
                    ps[hi][:, 512 : 512 + B * U],
                    bsum_sb[:, hi : hi + 1],
                ).then_inc(s_dve, 1)
            dve.tensor_copy(WjT_sb[:], WjT_stage[:]).then_inc(s_dve, 1)
            # s_dve = 9 after setup
            for rep in range(reps):
                for m in range(M):
                    c = rep * M + m
                    dve.wait_ge(s_pe, 8 + c + 1)
                    if c >= NOT:
                        dve.wait_ge(s_outd, 16 * ((c - NOT) + 1))  # ot slot free
                    dve.tensor_tensor(
                        ot_sb[:, c % NOT, :],
                        ps[c % NPS][:, :],
                        bj_sb[:, :],
                        mybir.AluOpType.add,
                    ).then_inc(s_dve, 1)

    return nc


def _tile_k(a):
    """[K, X] -> [128, K//128, X] with k = kt*128 + p."""
    k, x = a.shape
    return np.ascontiguousarray(a.reshape(k // 128, 128, x).transpose(1, 0, 2))


def _prep_inputs(enc_out, dec_out, W_enc, b_enc, W_dec, b_dec, W_joint, b_joint):
    enc_out = np.asarray(enc_out, dtype=np.float32)
    dec_out = np.asarray(dec_out, dtype=np.float32)
    common = {
        "decT": _tile_k(np.ascontiguousarray(dec_out.reshape(B * U, DEC_DIM).T)),
        "WencT": _tile_k(np.ascontiguousarray(np.asarray(W_enc, np.float32).T)),
        "WdecT": _tile_k(np.ascontiguousarray(np.asarray(W_dec, np.float32).T)),
        "WjT": _tile_k(np.ascontiguousarray(np.asarray(W_joint, np.float32).T)),
        "bsum": np.ascontiguousarray(
            (np.asarray(b_enc, np.float32) + np.asarray(b_dec, np.float32))
            .reshape(HT, 128)
            .T
        ),
        "bj": np.ascontiguousarray(
            np.broadcast_to(np.asarray(b_joint, np.float32), (128, VOCAB))
        ),
    }
    in_maps = []
    for i in range(NCORES):
        sl = enc_out[:, i * TC : (i + 1) * TC, :].reshape(M, ENC_DIM)
        m = dict(common)
        m["encT"] = _tile_k(np.ascontiguousarray(sl.T))
        in_maps.append(m)
    return in_maps


def run(in_maps, trace=False, **kw):
    from concourse.bass_utils import run_bass_kernel_spmd

    if "nc" not in _CACHE:
        _CACHE["nc"] = _build_bass()
    return run_bass_kernel_spmd(
        _CACHE["nc"], in_maps, list(range(NCORES)), trace=trace, **kw
    )


def time_kernel(in_maps, reps_list=(1, 9), n_meas=3):
    """HW time per main-loop pass via rep-count wall-clock deltas.

    Timing variants write to internal DRAM (tiny external output), so the
    axon transfer cost is identical across rep counts and cancels in the
    delta.
    """
    import time
    from concourse.bass_utils import run_bass_kernel_spmd

    walls = {}
    for reps in reps_list:
        key = f"t{reps}"
        if key not in _CACHE:
            _CACHE[key] = _build_bass(reps=reps, timing=True)
        nc = _CACHE[key]
        run_bass_kernel_spmd(nc, in_maps, list(range(NCORES)))  # compile+warm
        ts = []
        for _ in range(n_meas):
            t0 = time.time()
            run_bass_kernel_spmd(nc, in_maps, list(range(NCORES)))
            ts.append(time.time() - t0)
        walls[reps] = min(ts)
    r0, r1 = reps_list
    per_pass = (walls[r1] - walls[r0]) / (r1 - r0)
    return per_pass, walls


def kernel(enc_out, dec_out, W_enc, b_enc, W_dec, b_dec, W_joint, b_joint):
    import sys

    if "/opt/trn_rl_repo" not in sys.path:
        sys.path.insert(0, "/opt/trn_rl_repo")

    in_maps = _prep_inputs(
        enc_out, dec_out, W_enc, b_enc, W_dec, b_dec, W_joint, b_joint
    )
    res = run(in_maps)
    parts = [r["out"].reshape(B, TC, U, VOCAB) for r in res.results]
    return np.concatenate(parts, axis=1)



# revision 12
# speedup vs baseline: 655.5179x; 655.5179x over previous
"""RNN-T JointNetwork kernel for 8 Trainium2 NeuronCores (raw bass).

reference:
  e = enc @ W_enc.T + b_enc          # [B,T,H]
  d = dec @ W_dec.T + b_dec          # [B,U,H]
  j = tanh(e[:,:,None,:] + d[:,None,:,:])
  out = j @ W_joint.T + b_joint      # [B,T,U,V]

Sharding: T (256) split 8 ways -> 32 t-rows per core; host concatenates
along T.

Per-core dataflow (H on partitions for everything left of the big matmul):
  PE:  E^T[h, m] and D^T[h, n] projections, then per (b,t) row m the vocab
       matmul psum[u, v] += Jt[h, u]^T W_joint^T[h, v] (4 k-tiles x 2 v-banks)
  ACT: Jt[h, u] = tanh(D^T[h, (b,u)] + E^T[h, m]) via per-partition bias port
  DVE: drains psum -> sbuf while adding replicated b_joint
  SP:  all DMA (inputs + one 512KB contiguous output row per m)

The WHOLE pass (input DMA, projections, main loop) sits inside one
hardware Fori loop over `reps`. Semaphore-wait thresholds grow by a
constant amount per rep, so every wait site keeps one register per
(engine, semaphore) pair and advances it by a constant delta
(wait-then-add). The NEFF is therefore the same size for any rep count:
wall(reps_hi) - wall(reps_lo) isolates pure hardware execution time.

This toolchain's walrus rejects any compute instruction carrying >=2 sync
waits, so the kernel is written in raw bass: all cross-engine waits are
standalone wait_ge instructions and compute instructions carry none.

Sem accounting per rep (in-rep increments):
  s_in   +304 (19 input DMAs x16: 4x decT, 4x WdecT, encT, 4x WencT,
               bsum, 4x WjT, bj -- split so projections start as soon as
               their k-tile lands and the main loop as soon as WjT[h0] does)
  s_pe   +136 (4 D groups + 4 E groups + 128 main-matmul groups)
  s_act  +512 (4 tanh per m)
  s_dve  +136 (4 DT adds + 4 ET copies + 128 drains)
  s_outd +2048 (128 out-DMAs x16)
One-time bumps at program start: s_pe +3, s_dve +4, s_outd +128 so all
thresholds stay non-negative with uniform per-site deltas.

The projections run in plain f32: f32r tiles fed by raw DMA are only
accurate as the matmul's MOVING operand (WjT); a DMA-fed f32r STATIONARY
operand (tried for WdecT/WencT) produces large errors on hardware. The
known-good f32r tensors are WjT (moving, DMA-fed) and jt (stationary,
written on-chip by ACT).
"""

import numpy as np

B, T, U = 4, 256, 128
ENC_DIM = DEC_DIM = HID = 512
VOCAB = 1024
NCORES = 8
TC = T // NCORES        # 32 t-rows per core
M = B * TC              # 128 (b,t) rows per core
KT = HID // 128         # 4 contraction tiles
HT = HID // 128         # 4 h tiles
NJT = 4                 # jt double-buffers
NOT = 8                 # output staging buffers (must divide M)
NPS = 4                 # psum tiles in flight (8 banks total)

_CACHE = {}


def _build_bass(reps=1, timing=False):
    import concourse.bass as bass
    import concourse.mybir as mybir

    f32 = mybir.dt.float32
    f32r = mybir.dt.float32r
    Tanh = mybir.ActivationFunctionType.Tanh
    ET_ = mybir.EngineType

    nc = bass.Bass()
    encT = nc.declare_dram_parameter("encT", [128, KT, M], f32, isOutput=False)
    decT = nc.declare_dram_parameter("decT", [128, KT, B * U], f32, isOutput=False)
    WencT = nc.declare_dram_parameter("WencT", [128, KT, HID], f32, isOutput=False)
    WdecT = nc.declare_dram_parameter("WdecT", [128, KT, HID], f32, isOutput=False)
    WjT = nc.declare_dram_parameter("WjT", [128, HT, VOCAB], f32r, isOutput=False)
    bsum = nc.declare_dram_parameter("bsum", [128, HT], f32, isOutput=False)
    bj = nc.declare_dram_parameter("bj", [128, VOCAB], f32, isOutput=False)
    if timing:
        out = nc.dram_tensor("out_i", [M, U, VOCAB], f32)
        tok = nc.declare_dram_parameter("tok", [128, HT], f32, isOutput=True)
    else:
        out = nc.declare_dram_parameter("out", [M, U, VOCAB], f32, isOutput=True)

    from contextlib import ExitStack

    with ExitStack() as ctx:
        e = ctx.enter_context
        encT_sb = e(nc.sbuf_tensor("encT_sb", [128, KT, M], f32))
        decT_sb = e(nc.sbuf_tensor("decT_sb", [128, KT, B * U], f32))
        WencT_sb = e(nc.sbuf_tensor("WencT_sb", [128, KT, HID], f32))
        WdecT_sb = e(nc.sbuf_tensor("WdecT_sb", [128, KT, HID], f32))
        WjT_sb = e(nc.sbuf_tensor("WjT_sb", [128, HT, VOCAB], f32r))
        bsum_sb = e(nc.sbuf_tensor("bsum_sb", [128, HT], f32))
        bj_sb = e(nc.sbuf_tensor("bj_sb", [128, VOCAB], f32))
        ET_sb = e(nc.sbuf_tensor("ET_sb", [128, HT, M], f32))
        DT_sb = e(nc.sbuf_tensor("DT_sb", [128, HT, B * U], f32))
        jt_sb = e(nc.sbuf_tensor("jt_sb", [128, NJT, HT, 128], f32r))
        ot_sb = e(nc.sbuf_tensor("ot_sb", [128, NOT, VOCAB], f32))
        ps = [
            e(nc.psum_tensor(f"ps{i}", [128, VOCAB], f32)) for i in range(NPS)
        ]
        s_in = e(nc.semaphore("s_in"))
        s_act = e(nc.semaphore("s_act"))
        s_pe = e(nc.semaphore("s_pe"))
        s_dve = e(nc.semaphore("s_dve"))
        s_outd = e(nc.semaphore("s_outd"))

        SP = nc.sync
        PE = nc.tensor
        ACT = nc.scalar
        DVE = nc.vector

        # --- one-time bumps so every per-site threshold is >= 0 ----------
        SP.sem_inc(s_pe, 3)
        SP.sem_inc(s_dve, 4)
        SP.sem_inc(s_outd, 128)

        # --- per-(engine, sem) cumulative wait-threshold registers -------
        def reg(eng, name, init):
            r = eng.alloc_register(name)
            eng.reg_mov(r, init)
            return r

        r_sd = reg(SP, "r_sd", 13)    # SP waits s_dve (drain m done)
        r_pi = reg(PE, "r_pi", 32)    # PE waits s_in (staged input arrival)
        r_pd = reg(PE, "r_pd", 1)     # PE waits s_dve (psum slot free)
        r_pa = reg(PE, "r_pa", 4)     # PE waits s_act (jt ready)
        r_ad = reg(ACT, "r_ad", 12)   # ACT waits s_dve (ET/DT ready)
        r_ap = reg(ACT, "r_ap", 0)    # ACT waits s_pe (jt slot free)
        r_va = reg(DVE, "r_va", 0)    # DVE waits s_act (ET/DT overwrite ok)
        r_vi = reg(DVE, "r_vi", 224)  # DVE waits s_in (bsum, then bj)
        r_vp = reg(DVE, "r_vp", 4)    # DVE waits s_pe (psum group done)
        r_vo = reg(DVE, "r_vo", 16)   # DVE waits s_outd (ot slot free)

        def site(eng, r, sem, delta):
            eng.wait_ge(sem, r)
            if delta:
                eng.reg_add(r, r, delta)

        with nc.Fori(0, reps, engines=[ET_.SP, ET_.PE, ET_.Activation, ET_.DVE]):
            # ---------------- SP: DMAs ----------------
            # Input DMAs: the previous rep's m=127 site already guaranteed
            # all compute of that rep finished, so overwriting is safe.
            # Order = consumption order; each lands 16 on s_in:
            #   (decT_k, WdecT_k) pairs: 32*(k+1)    -> D^T k-tile gates
            #   encT: 144, WencT_k: 160+16k          -> E^T k-tile gates
            #   bsum: 224                            -> DT adds
            #   WjT_h: 240+16h                       -> main matmul h-panels
            #   bj: 304                              -> drains
            for ki in range(KT):
                SP.dma_start(out=decT_sb[:, ki, :], in_=decT[:, ki, :]).then_inc(
                    s_in, 16
                )
                SP.dma_start(out=WdecT_sb[:, ki, :], in_=WdecT[:, ki, :]).then_inc(
                    s_in, 16
                )
            SP.dma_start(out=encT_sb[:], in_=encT[:]).then_inc(s_in, 16)
            for ki in range(KT):
                SP.dma_start(out=WencT_sb[:, ki, :], in_=WencT[:, ki, :]).then_inc(
                    s_in, 16
                )
            SP.dma_start(out=bsum_sb[:], in_=bsum[:]).then_inc(s_in, 16)
            for hi in range(HT):
                SP.dma_start(out=WjT_sb[:, hi, :], in_=WjT[:, hi, :]).then_inc(
                    s_in, 16
                )
            SP.dma_start(out=bj_sb[:], in_=bj[:]).then_inc(s_in, 16)
            for m in range(M):
                site(SP, r_sd, s_dve, 1 if m < M - 1 else 9)
                SP.dma_start(out=out[m], in_=ot_sb[:, m % NOT, :]).then_inc(
                    s_outd, 16
                )

            # ---------------- PE: matmuls ----------------
            # D^T[h, n] projections (f32r, k-outer): ps[hi][:, 512:1024]
            for ki in range(KT):
                site(PE, r_pi, s_in, 32)
                for hi in range(HT):
                    if ki == 0:
                        site(PE, r_pd, s_dve, 1 if hi < HT - 1 else 9 - HT)
                    mm = PE.matmul(
                        ps[hi][:, 512 : 512 + B * U],
                        WdecT_sb[:, ki, hi * 128 : (hi + 1) * 128],
                        decT_sb[:, ki, :],
                        start=(ki == 0),
                        stop=(ki == KT - 1),
                    )
                    if ki == KT - 1:
                        mm.then_inc(s_pe, 1)
            # E^T[h, m] projections (f32, k-outer): ps[hi][:, 0:M]
            for ki in range(KT):
                site(PE, r_pi, s_in, 32 if ki == KT - 1 else 16)
                for hi in range(HT):
                    mm = PE.matmul(
                        ps[hi][:, 0:M],
                        WencT_sb[:, ki, hi * 128 : (hi + 1) * 128],
                        encT_sb[:, ki, :],
                        start=(ki == 0),
                        stop=(ki == KT - 1),
                    )
                    if ki == KT - 1:
                        mm.then_inc(s_pe, 1)
            # main loop
            for m in range(M):
                site(PE, r_pa, s_act, 4)
                site(PE, r_pd, s_dve, 1)
                for hi in range(HT):
                    if m == 0:
                        site(PE, r_pi, s_in, 48 if hi == HT - 1 else 16)
                    for vi in range(2):
                        mm = PE.matmul(
                            ps[m % NPS][:, vi * 512 : (vi + 1) * 512],
                            jt_sb[:, m % NJT, hi, :],
                            WjT_sb[:, hi, vi * 512 : (vi + 1) * 512],
                            start=(hi == 0),
                            stop=(hi == HT - 1),
                        )
                mm.then_inc(s_pe, 1)

            # ---------------- ACT: tanh ----------------
            site(ACT, r_ad, s_dve, 136)
            for m in range(M):
                b = m // TC
                site(ACT, r_ap, s_pe, 9 if m == 3 else 1)
                for hi in range(HT):
                    ACT.activation(
                        jt_sb[:, m % NJT, hi, :],
                        DT_sb[:, hi, b * 128 : (b + 1) * 128],
                        Tanh,
                        bias=ET_sb[:, hi, m : m + 1],
                    ).then_inc(s_act, 1)

            # ---------------- DVE: copies / adds / drains ----------------
            site(DVE, r_va, s_act, 512)
            site(DVE, r_vi, s_in, 80)
            for hi in range(HT):
                site(DVE, r_vp, s_pe, 1)
                DVE.tensor_scalar_add(
                    DT_sb[:, hi, :],
                    ps[hi][:, 512 : 512 + B * U],
                    bsum_sb[:, hi : hi + 1],
                ).then_inc(s_dve, 1)
            for hi in range(HT):
                site(DVE, r_vp, s_pe, 1)
                DVE.tensor_copy(ET_sb[:, hi, :], ps[hi][:, 0:M]).then_inc(s_dve, 1)
            site(DVE, r_vi, s_in, 224)
            for m in range(M):
                site(DVE, r_vp, s_pe, 1)
                site(DVE, r_vo, s_outd, 16)
                DVE.tensor_tensor(
                    ot_sb[:, m % NOT, :],
                    ps[m % NPS][:, :],
                    bj_sb[:, :],
                    mybir.AluOpType.add,
                ).then_inc(s_dve, 1)

        SP.wait_ge(s_outd, 128 + 16 * M * reps)
        if timing:
            SP.dma_start(out=tok[:], in_=bsum_sb[:]).then_inc(s_in, 16)
            SP.wait_ge(s_in, 304 * reps + 16)
        nc.all_engine_barrier()

    return nc


def _tile_k(a):
    """[K, X] -> [128, K//128, X] with k = kt*128 + p."""
    k, x = a.shape
    return np.ascontiguousarray(a.reshape(k // 128, 128, x).transpose(1, 0, 2))


def _prep_inputs(enc_out, dec_out, W_enc, b_enc, W_dec, b_dec, W_joint, b_joint):
    enc_out = np.asarray(enc_out, dtype=np.float32)
    dec_out = np.asarray(dec_out, dtype=np.float32)
    common = {
        "decT": _tile_k(np.ascontiguousarray(dec_out.reshape(B * U, DEC_DIM).T)),
        "WencT": _tile_k(np.ascontiguousarray(np.asarray(W_enc, np.float32).T)),
        "WdecT": _tile_k(np.ascontiguousarray(np.asarray(W_dec, np.float32).T)),
        "WjT": _tile_k(np.ascontiguousarray(np.asarray(W_joint, np.float32).T)),
        "bsum": np.ascontiguousarray(
            (np.asarray(b_enc, np.float32) + np.asarray(b_dec, np.float32))
            .reshape(HT, 128)
            .T
        ),
        "bj": np.ascontiguousarray(
            np.broadcast_to(np.asarray(b_joint, np.float32), (128, VOCAB))
        ),
    }
    in_maps = []
    for i in range(NCORES):
        sl = enc_out[:, i * TC : (i + 1) * TC, :].reshape(M, ENC_DIM)
        m = dict(common)
        m["encT"] = _tile_k(np.ascontiguousarray(sl.T))
        in_maps.append(m)
    return in_maps


def run(in_maps, trace=False, **kw):
    from concourse.bass_utils import run_bass_kernel_spmd

    if "nc" not in _CACHE:
        _CACHE["nc"] = _build_bass()
    return run_bass_kernel_spmd(
        _CACHE["nc"], in_maps, list(range(NCORES)), trace=trace, **kw
    )


def time_kernel(in_maps, reps_list=(8, 264), n_meas=3):
    """Pure-HW time per pass via rep-count wall-clock deltas.

    The whole pass runs inside a hardware Fori loop, so the two NEFFs are
    byte-identical except for one loop-bound immediate: host-side
    compile/serialization/transfer cost is identical across rep counts and
    cancels exactly in the delta.
    """
    import time
    from concourse.bass_utils import run_bass_kernel_spmd

    walls = {}
    for reps in reps_list:
        key = f"t{reps}"
        if key not in _CACHE:
            _CACHE[key] = _build_bass(reps=reps, timing=True)
        nc = _CACHE[key]
        run_bass_kernel_spmd(nc, in_maps, list(range(NCORES)))  # compile+warm
        ts = []
        for _ in range(n_meas):
            t0 = time.time()
            run_bass_kernel_spmd(nc, in_maps, list(range(NCORES)))
            ts.append(time.time() - t0)
        walls[reps] = min(ts)
    r0, r1 = reps_list
    per_pass = (walls[r1] - walls[r0]) / (r1 - r0)
    return per_pass, walls


def kernel(enc_out, dec_out, W_enc, b_enc, W_dec, b_dec, W_joint, b_joint):
    import sys

    if "/opt/trn_rl_repo" not in sys.path:
        sys.path.insert(0, "/opt/trn_rl_repo")

    in_maps = _prep_inputs(
        enc_out, dec_out, W_enc, b_enc, W_dec, b_dec, W_joint, b_joint
    )
    res = run(in_maps)
    parts = [r["out"].reshape(B, TC, U, VOCAB) for r in res.results]
    return np.concatenate(parts, axis=1)


# revision 29
# speedup vs baseline: 875.2224x; 1.3352x over previous
"""RNN-T JointNetwork kernel for 8 Trainium2 NeuronCores (raw bass).

reference:
  e = enc @ W_enc.T + b_enc          # [B,T,H]
  d = dec @ W_dec.T + b_dec          # [B,U,H]
  j = tanh(e[:,:,None,:] + d[:,None,:,:])
  out = j @ W_joint.T + b_joint      # [B,T,U,V]

Sharding: T (256) split 8 ways -> 32 t-rows per core; host concatenates
along T.

Per-core dataflow (H on partitions for everything left of the big matmul):
  PE:  E^T[h, m] and D^T[h, n] projections, then per (b,t) row m the vocab
       matmul psum[u, v] += Jt[h, u]^T W_joint^T[h, v] (4 k-tiles x 2 v-banks)
  ACT: Jt[h, u] = tanh(D^T[h, (b,u)] + E^T[h, m]) via per-partition bias port
  DVE: drains psum -> sbuf while adding replicated b_joint
  SP:  all DMA (inputs + one 512KB contiguous output row per m)

The WHOLE pass (input DMA, projections, main loop) sits inside one
hardware Fori loop over `reps`. Semaphore-wait thresholds grow by a
constant amount per rep, so every wait site keeps one register per
(engine, semaphore) pair and advances it by a constant delta
(wait-then-add). The NEFF is therefore the same size for any rep count:
wall(reps_hi) - wall(reps_lo) isolates pure hardware execution time.

This toolchain's walrus rejects any compute instruction carrying >=2 sync
waits, so the kernel is written in raw bass: all cross-engine waits are
standalone wait_ge instructions and compute instructions carry none.

Sem accounting per rep (in-rep increments):
  s_in   +304 (19 input DMAs x16: 4x decT, 4x WdecT, encT, 4x WencT,
               bsum, 4x WjT, bj -- split so projections start as soon as
               their k-tile lands and the main loop as soon as WjT[h0] does)
  s_pe   +136 (4 D groups + 4 E groups + 128 main-matmul groups)
  s_act  +512 (4 tanh per m)
  s_dve  +136 (4 DT adds + 4 ET copies + 128 drains)
  s_outd +2048 (128 out-DMAs x16)
One-time bumps at program start: s_pe +7, s_dve +4, s_outd +256 so all
thresholds stay non-negative with uniform per-site deltas.

The main matmul runs in bf16 (jt written bf16 by ACT, WjT converted on
the host): bf16 streams the PE at the full 1 cycle/row rate, while f32
takes 4 and f32r measured ~1.5 on hardware. The projections stay plain
f32 -- bf16 (and f32r) DMA-fed projection operands produced large errors
on hardware in the full kernel even though isolated probes of the same
slicing passed. PSUM accumulation, biases, tanh inputs and the f32
output are unaffected; measured error is 2.3e-3 Frobenius vs the 2e-2
gate.
"""

import numpy as np

B, T, U = 4, 256, 128
ENC_DIM = DEC_DIM = HID = 512
VOCAB = 1024
NCORES = 8
TC = T // NCORES        # 32 t-rows per core
M = B * TC              # 128 (b,t) rows per core
KT = HID // 128         # 4 contraction tiles
HT = HID // 128         # 4 h tiles
NJT = 8                 # jt buffers (ACT can run 8 rows ahead)
NOT = 16                # output staging buffers (must divide M)
NPS = 4                 # psum tiles in flight (8 banks total)

_CACHE = {}


def _build_bass(reps=1, timing=False):
    import concourse.bass as bass
    import concourse.mybir as mybir

    f32 = mybir.dt.float32
    bf16 = mybir.dt.bfloat16
    Tanh = mybir.ActivationFunctionType.Tanh
    ET_ = mybir.EngineType

    nc = bass.Bass()
    encT = nc.declare_dram_parameter("encT", [128, KT, M], f32, isOutput=False)
    decT = nc.declare_dram_parameter("decT", [128, KT, B * U], f32, isOutput=False)
    WencT = nc.declare_dram_parameter("WencT", [128, KT, HID], f32, isOutput=False)
    WdecT = nc.declare_dram_parameter("WdecT", [128, KT, HID], f32, isOutput=False)
    WjT = nc.declare_dram_parameter("WjT", [128, HT, VOCAB], bf16, isOutput=False)
    bsum = nc.declare_dram_parameter("bsum", [128, HT], f32, isOutput=False)
    bj = nc.declare_dram_parameter("bj", [128, VOCAB], f32, isOutput=False)
    if timing:
        out = nc.dram_tensor("out_i", [M, U, VOCAB], f32)
        tok = nc.declare_dram_parameter("tok", [128, HT], f32, isOutput=True)
    else:
        out = nc.declare_dram_parameter("out", [M, U, VOCAB], f32, isOutput=True)

    from contextlib import ExitStack

    with ExitStack() as ctx:
        e = ctx.enter_context
        encT_sb = e(nc.sbuf_tensor("encT_sb", [128, KT, M], f32))
        decT_sb = e(nc.sbuf_tensor("decT_sb", [128, KT, B * U], f32))
        WencT_sb = e(nc.sbuf_tensor("WencT_sb", [128, KT, HID], f32))
        WdecT_sb = e(nc.sbuf_tensor("WdecT_sb", [128, KT, HID], f32))
        WjT_sb = e(nc.sbuf_tensor("WjT_sb", [128, HT, VOCAB], bf16))
        bsum_sb = e(nc.sbuf_tensor("bsum_sb", [128, HT], f32))
        bj_sb = e(nc.sbuf_tensor("bj_sb", [128, VOCAB], f32))
        ET_sb = e(nc.sbuf_tensor("ET_sb", [128, HT, M], f32))
        DT_sb = e(nc.sbuf_tensor("DT_sb", [128, HT, B * U], f32))
        jt_sb = e(nc.sbuf_tensor("jt_sb", [128, NJT, HT, 128], bf16))
        ot_sb = e(nc.sbuf_tensor("ot_sb", [128, NOT, VOCAB], f32))
        ps = [
            e(nc.psum_tensor(f"ps{i}", [128, VOCAB], f32)) for i in range(NPS)
        ]
        s_in = e(nc.semaphore("s_in"))
        s_act = e(nc.semaphore("s_act"))
        s_pe = e(nc.semaphore("s_pe"))
        s_dve = e(nc.semaphore("s_dve"))
        s_outd = e(nc.semaphore("s_outd"))

        SP = nc.sync
        PE = nc.tensor
        ACT = nc.scalar
        DVE = nc.vector

        # --- one-time bumps so every per-site threshold is >= 0 ----------
        SP.sem_inc(s_pe, 7)
        SP.sem_inc(s_dve, 4)
        SP.sem_inc(s_outd, 256)

        # --- per-(engine, sem) cumulative wait-threshold registers -------
        def reg(eng, name, init):
            r = eng.alloc_register(name)
            eng.reg_mov(r, init)
            return r

        r_sd = reg(SP, "r_sd", 13)    # SP waits s_dve (drain m done)
        r_pi = reg(PE, "r_pi", 32)    # PE waits s_in (staged input arrival)
        r_pd = reg(PE, "r_pd", 1)     # PE waits s_dve (psum slot free)
        r_pa = reg(PE, "r_pa", 4)     # PE waits s_act (jt ready)
        r_ad = reg(ACT, "r_ad", 12)   # ACT waits s_dve (ET/DT ready)
        r_ap = reg(ACT, "r_ap", 0)    # ACT waits s_pe (jt slot free)
        r_va = reg(DVE, "r_va", 0)    # DVE waits s_act (ET/DT overwrite ok)
        r_vi = reg(DVE, "r_vi", 224)  # DVE waits s_in (bsum, then bj)
        r_vp = reg(DVE, "r_vp", 8)    # DVE waits s_pe (psum group done)
        r_vo = reg(DVE, "r_vo", 16)   # DVE waits s_outd (ot slot free)

        def site(eng, r, sem, delta):
            eng.wait_ge(sem, r)
            if delta:
                eng.reg_add(r, r, delta)

        with nc.Fori(0, reps, engines=[ET_.SP, ET_.PE, ET_.Activation, ET_.DVE]):
            # ---------------- SP: DMAs ----------------
            # Input DMAs: the previous rep's m=127 site already guaranteed
            # all compute of that rep finished, so overwriting is safe.
            # Order = consumption order; each lands 16 on s_in:
            #   (decT_k, WdecT_k) pairs: 32*(k+1)    -> D^T k-tile gates
            #   encT: 144, WencT_k: 160+16k          -> E^T k-tile gates
            #   bsum: 224                            -> DT adds
            #   WjT_h: 240+16h                       -> main matmul h-panels
            #   bj: 304                              -> drains
            for ki in range(KT):
                SP.dma_start(out=decT_sb[:, ki, :], in_=decT[:, ki, :]).then_inc(
                    s_in, 16
                )
                SP.dma_start(out=WdecT_sb[:, ki, :], in_=WdecT[:, ki, :]).then_inc(
                    s_in, 16
                )
            SP.dma_start(out=encT_sb[:], in_=encT[:]).then_inc(s_in, 16)
            for ki in range(KT):
                SP.dma_start(out=WencT_sb[:, ki, :], in_=WencT[:, ki, :]).then_inc(
                    s_in, 16
                )
            SP.dma_start(out=bsum_sb[:], in_=bsum[:]).then_inc(s_in, 16)
            for hi in range(HT):
                SP.dma_start(out=WjT_sb[:, hi, :], in_=WjT[:, hi, :]).then_inc(
                    s_in, 16
                )
            SP.dma_start(out=bj_sb[:], in_=bj[:]).then_inc(s_in, 16)
            for m in range(M):
                site(SP, r_sd, s_dve, 1 if m < M - 1 else 9)
                SP.dma_start(out=out[m], in_=ot_sb[:, m % NOT, :]).then_inc(
                    s_outd, 16
                )

            # ---------------- PE: matmuls ----------------
            # D^T[h, n] projections (k-outer): ps[hi][:, 512:1024]
            for ki in range(KT):
                site(PE, r_pi, s_in, 32)
                for hi in range(HT):
                    if ki == 0:
                        site(PE, r_pd, s_dve, 1 if hi < HT - 1 else 5)
                    mm = PE.matmul(
                        ps[hi][:, 512 : 512 + B * U],
                        WdecT_sb[:, ki, hi * 128 : (hi + 1) * 128],
                        decT_sb[:, ki, :],
                        start=(ki == 0),
                        stop=(ki == KT - 1),
                    )
                    if ki == KT - 1:
                        mm.then_inc(s_pe, 1)
            # E^T[h, m] projections (k-outer): ps[hi][:, 0:M]
            for ki in range(KT):
                site(PE, r_pi, s_in, 32 if ki == KT - 1 else 16)
                for hi in range(HT):
                    mm = PE.matmul(
                        ps[hi][:, 0:M],
                        WencT_sb[:, ki, hi * 128 : (hi + 1) * 128],
                        encT_sb[:, ki, :],
                        start=(ki == 0),
                        stop=(ki == KT - 1),
                    )
                    if ki == KT - 1:
                        mm.then_inc(s_pe, 1)
            # main loop
            for m in range(M):
                site(PE, r_pa, s_act, 4)
                site(PE, r_pd, s_dve, 1)
                for hi in range(HT):
                    if m == 0:
                        site(PE, r_pi, s_in, 48 if hi == HT - 1 else 16)
                    for vi in range(2):
                        mm = PE.matmul(
                            ps[m % NPS][:, vi * 512 : (vi + 1) * 512],
                            jt_sb[:, m % NJT, hi, :],
                            WjT_sb[:, hi, vi * 512 : (vi + 1) * 512],
                            start=(hi == 0),
                            stop=(hi == HT - 1),
                        )
                mm.then_inc(s_pe, 1)

            # ---------------- ACT: tanh ----------------
            site(ACT, r_ad, s_dve, 136)
            for m in range(M):
                b = m // TC
                site(ACT, r_ap, s_pe, 9 if m == 7 else 1)
                for hi in range(HT):
                    ACT.activation(
                        jt_sb[:, m % NJT, hi, :],
                        DT_sb[:, hi, b * 128 : (b + 1) * 128],
                        Tanh,
                        bias=ET_sb[:, hi, m : m + 1],
                    ).then_inc(s_act, 1)

            # ---------------- DVE: copies / adds / drains ----------------
            site(DVE, r_va, s_act, 512)
            site(DVE, r_vi, s_in, 80)
            for hi in range(HT):
                site(DVE, r_vp, s_pe, 1)
                DVE.tensor_scalar_add(
                    DT_sb[:, hi, :],
                    ps[hi][:, 512 : 512 + B * U],
                    bsum_sb[:, hi : hi + 1],
                ).then_inc(s_dve, 1)
            for hi in range(HT):
                site(DVE, r_vp, s_pe, 1)
                DVE.tensor_copy(ET_sb[:, hi, :], ps[hi][:, 0:M]).then_inc(s_dve, 1)
            site(DVE, r_vi, s_in, 224)
            for m in range(M):
                site(DVE, r_vp, s_pe, 1)
                site(DVE, r_vo, s_outd, 16)
                DVE.tensor_tensor(
                    ot_sb[:, m % NOT, :],
                    ps[m % NPS][:, :],
                    bj_sb[:, :],
                    mybir.AluOpType.add,
                ).then_inc(s_dve, 1)

        SP.wait_ge(s_outd, 256 + 16 * M * reps)
        if timing:
            SP.dma_start(out=tok[:], in_=bsum_sb[:]).then_inc(s_in, 16)
            SP.wait_ge(s_in, 304 * reps + 16)
        nc.all_engine_barrier()

    return nc


def _tile_k(a):
    """[K, X] -> [128, K//128, X] with k = kt*128 + p."""
    k, x = a.shape
    return np.ascontiguousarray(a.reshape(k // 128, 128, x).transpose(1, 0, 2))


def _prep_inputs(enc_out, dec_out, W_enc, b_enc, W_dec, b_dec, W_joint, b_joint):
    import ml_dtypes

    bf16 = ml_dtypes.bfloat16
    enc_out = np.asarray(enc_out, dtype=np.float32)
    dec_out = np.asarray(dec_out, dtype=np.float32)
    common = {
        "decT": _tile_k(np.ascontiguousarray(dec_out.reshape(B * U, DEC_DIM).T)),
        "WencT": _tile_k(np.ascontiguousarray(np.asarray(W_enc, np.float32).T)),
        "WdecT": _tile_k(np.ascontiguousarray(np.asarray(W_dec, np.float32).T)),
        "WjT": _tile_k(np.ascontiguousarray(np.asarray(W_joint, np.float32).T.astype(bf16))),
        "bsum": np.ascontiguousarray(
            (np.asarray(b_enc, np.float32) + np.asarray(b_dec, np.float32))
            .reshape(HT, 128)
            .T
        ),
        "bj": np.ascontiguousarray(
            np.broadcast_to(np.asarray(b_joint, np.float32), (128, VOCAB))
        ),
    }
    in_maps = []
    for i in range(NCORES):
        sl = enc_out[:, i * TC : (i + 1) * TC, :].reshape(M, ENC_DIM)
        m = dict(common)
        m["encT"] = _tile_k(np.ascontiguousarray(sl.T))
        in_maps.append(m)
    return in_maps


def run(in_maps, trace=False, **kw):
    from concourse.bass_utils import run_bass_kernel_spmd

    if "nc" not in _CACHE:
        _CACHE["nc"] = _build_bass()
    return run_bass_kernel_spmd(
        _CACHE["nc"], in_maps, list(range(NCORES)), trace=trace, **kw
    )


def _make_runner(nc, in_maps):
    """Persistent jitted executor for a timing NEFF.

    Unlike run_bass_kernel_spmd (which re-traces, re-serializes the BIR and
    re-uploads all inputs on every call), this device_puts the inputs once
    and caches the jitted shard_map, so each call() is just dispatch + the
    NEFF's on-device execution.
    """
    import jax
    import numpy as np
    from jax.sharding import Mesh, NamedSharding, PartitionSpec
    from jax.experimental.shard_map import shard_map
    from concourse import bass2jax
    import concourse.mybir as mybir

    bass2jax.install_neuronx_cc_hook()
    partition_name = (
        nc.partition_id_tensor.name if nc.partition_id_tensor else None
    )
    in_names, out_names, out_avals, zero_outs = [], [], [], []
    for alloc in nc.m.functions[0].allocations:
        if not isinstance(alloc, mybir.MemoryLocationSet):
            continue
        name = alloc.memorylocations[0].name
        if alloc.kind == "ExternalInput":
            if name != partition_name:
                in_names.append(name)
        elif alloc.kind == "ExternalOutput":
            shape = tuple(alloc.tensor_shape)
            dtype = mybir.dt.np(alloc.dtype)
            out_names.append(name)
            out_avals.append(jax.core.ShapedArray(shape, dtype))
            zero_outs.append(np.zeros(shape, dtype))
    n_params = len(in_names)
    all_names = tuple(in_names) + tuple(out_names)
    if partition_name is not None:
        all_names = all_names + (partition_name,)

    def _body(*args):
        operands = list(args)
        if partition_name is not None:
            operands.append(bass2jax.partition_id_tensor())
        return tuple(
            bass2jax._bass_exec_p.bind(
                *operands,
                out_avals=tuple(out_avals),
                in_names=all_names,
                out_names=tuple(out_names),
                lowering_input_output_aliases=(),
                sim_require_finite=True,
                sim_require_nnan=True,
                nc=nc,
            )
        )

    devices = jax.devices()[:NCORES]
    mesh = Mesh(np.asarray(devices), ("core",))
    nspec = NamedSharding(mesh, PartitionSpec("core"))
    sharded = jax.jit(
        shard_map(
            _body,
            mesh=mesh,
            in_specs=(PartitionSpec("core"),) * (n_params + len(out_names)),
            out_specs=(PartitionSpec("core"),) * len(out_names),
            check_rep=False,
        )
    )
    dev_in = [
        jax.device_put(
            np.concatenate(
                [np.asarray(in_maps[c][n]) for c in range(NCORES)], axis=0
            ),
            nspec,
        )
        for n in in_names
    ]
    dev_zero = [
        jax.device_put(np.zeros((NCORES * z.shape[0], *z.shape[1:]), z.dtype), nspec)
        for z in zero_outs
    ]

    def call():
        jax.block_until_ready(sharded(*dev_in, *dev_zero))

    call()  # compile + warm
    return call


def time_kernel(in_maps, reps_list=(8, 1032), n_rounds=8):
    """Pure-HW time per pass via rep-count wall-clock deltas.

    The whole pass runs inside a hardware Fori loop, so the two NEFFs are
    byte-identical except for one loop-bound immediate: everything host-side
    is identical across rep counts and cancels in the delta. Runners keep
    inputs device-resident and the executable cached, so each call is
    dispatch + device execution; rounds are interleaved and we take the
    per-variant minimum to reject host noise.
    """
    import time

    runners = {}
    for reps in reps_list:
        key = f"t{reps}"
        if key not in _CACHE:
            _CACHE[key] = _build_bass(reps=reps, timing=True)
        runners[reps] = _make_runner(_CACHE[key], in_maps)
    walls = {r: [] for r in reps_list}
    for _ in range(n_rounds):
        for reps in reps_list:
            t0 = time.time()
            runners[reps]()
            walls[reps].append(time.time() - t0)
    mins = {r: min(w) for r, w in walls.items()}
    r0, r1 = reps_list
    per_pass = (mins[r1] - mins[r0]) / (r1 - r0)
    return per_pass, mins


def kernel(enc_out, dec_out, W_enc, b_enc, W_dec, b_dec, W_joint, b_joint):
    import sys

    if "/opt/trn_rl_repo" not in sys.path:
        sys.path.insert(0, "/opt/trn_rl_repo")

    in_maps = _prep_inputs(
        enc_out, dec_out, W_enc, b_enc, W_dec, b_dec, W_joint, b_joint
    )
    res = run(in_maps)
    parts = [r["out"].reshape(B, TC, U, VOCAB) for r in res.results]
    return np.concatenate(parts, axis=1)
